# revision 69
# baseline (speedup 1.0000x reference)
"""Trainium2 Bass kernel for the VQ commitment-loss problem (fp8 DoubleRow).

Math
----
reference loss = 0.25 * mean((codebook[argmin_k dist] - flat)**2)
               = 0.25/(B*T*D) * sum_n min_k ||flat_n - e_k||^2
since the gathered quantized row realizes exactly the min squared distance.

min_k ||f - e||^2 = ||f||^2 + min_k (||e_k||^2 - 2 f.e_k)

The ||f||^2 term is a tiny O(B*P*T) reduction of the (fp8-rounded) input,
computed on the host via the window-count trick.  The device computes only
the dominant O(N*K*D) term: per core (2 of 16 batches)

  - sum_n min_k (||e_k||^2 - 2 f_n.e_k) via fp8e4 DoubleRow TensorE matmuls
    (256-deep contraction per pass): window tiles [128, 4sub, T] are the
    stationary operand, the codebook scaled by -2 is the moving operand
    (two [128, 4sub, 512] code-half tiles).  ||e_k||^2 rides as three extra
    contraction rows (32*r0 + r1 + r2 fp8 decomposition, precomputed on the
    host, paired with a [32,1,1,1,1] column in the window operand).
  - each code-half accumulates into its OWN 2-bank PSUM tile so every
    matmul region has exactly one drain reader and the write-after-read
    wait of the pair-after-next's matmuls is a single fast semaphore
    (engines can wait on only one semaphore per instruction, so shared
    tiles force Tile to serialize the whole drain into the PE path).
  - drain per pair (PE pace ~1.73us): ScalarE converts the h1 half to
    fp16 (~1.11us) in parallel with a VectorE free-axis min-reduce of
    the h0 half straight out of PSUM (~1.23us); VectorE then folds the
    fp16 half once (~0.42us) and the 257-wide partials ship to DRAM on
    the otherwise-idle sync HWDGE ring.  The host finishes the last min
    levels and the sum (order-invariant).

DMA: the host pre-expands the ENTIRE im2col window tensor (dense,
contiguous per-partition lines) so the device needs only a handful of
large transfers on the gpsimd SWDGE queue (16 SDMA engines) instead of
descriptor-heavy strided expansion; the subtile processing order
(b0[0:2048), b1[0:2048), b0 tail, b1 tail) matches the FIFO landing
order so every transfer has a wide deadline.  N=128 warmup matmuls
bridge the PE HAM clock to 2.4 GHz until the first operands land.

Host side pads/casts/shards inputs, precomputes the codebook norm rows,
the dense window tensor and the ||f||^2 self term, and reduces the
per-core partials.
"""

import numpy as np
import ml_dtypes

B, P, T = 16, 12, 4096
WIN = 41
PAD = (WIN - 1) // 2          # 20
K = 1024
D = P * WIN                   # 492
COMMITMENT_COST = 0.25

NCORES = 8
BC = B // NCORES              # batches per core = 2
TP = T + 2 * PAD              # padded time = 4136
NCHUNK = 4                    # contraction subtiles: 3 pellets * 41 taps = 123 rows
CHROWS = 3 * WIN              # 123
NSUB = BC * T // 128          # 64 subtiles of 128 windows per core
NWARM = 34                    # HAM warmup matmuls (bridge PE to main-loop start)

SCALE = COMMITMENT_COST / (B * T * D)

FP8NP = ml_dtypes.float8_e4m3

_CACHED = {}


def _build_nc():
    import concourse.bacc as bacc
    import concourse.bass as bass
    import concourse.mybir as mybir
    import concourse.tile as tile

    BF = mybir.dt.bfloat16
    F32 = mybir.dt.float32
    F16 = mybir.dt.float16
    F8 = mybir.dt.float8e4
    OP = mybir.AluOpType
    ACT = mybir.ActivationFunctionType
    DR = mybir.MatmulPerfMode.DoubleRow

    nc = bacc.Bacc("TRN2", target_bir_lowering=False, debug=False)

    # host pre-expanded window tensors for this core's two batches; the head
    # columns live in their own tensor so the first DMA has a fully
    # contiguous destination tile (large descriptors, fast landing)
    xh_d = nc.dram_tensor("xh", [128, NCHUNK, 256], F8, kind="ExternalInput")
    xm_d = nc.dram_tensor("xm", [128, NCHUNK, 1792], F8, kind="ExternalInput")
    xg_d = nc.dram_tensor("xg", [128, NCHUNK, 2048], F8, kind="ExternalInput")
    x1_d = nc.dram_tensor("x1", [128, NCHUNK, T], F8, kind="ExternalInput")
    cb_d = nc.dram_tensor("cb", [2, 128, NCHUNK, 512], F8, kind="ExternalInput")
    # per-pair partials [pair, window, subtile, 257]: cols 0:256 are the
    # once-folded fp16 h1 distances, col 256 is the h0 min; host finishes
    # the reduction + the sum
    outw_d = nc.dram_tensor("outw", [NSUB // 2, 128, 2, 257], F16, kind="ExternalOutput")

    with tile.TileContext(nc) as tc:
        with (
            tc.tile_pool(name="cbpool", bufs=1) as cbpool,
            tc.tile_pool(name="wpool", bufs=1) as wpool,
            tc.tile_pool(name="misc", bufs=1) as misc,
        ):
            # resident codebook code-half tiles [k, chunk, code]; rows 123..125
            # of chunk 0 carry the host ||e||^2 fp8 decomposition
            cbt = [
                cbpool.tile([128, NCHUNK, 512], F8, tag=f"cb{h}", name=f"cbt{h}")
                for h in range(2)
            ]
            # resident window tiles wt[b]: [128, chunk, T] fp8 with
            # wt[b][k, c, t] = xw[b, 3c + k//41, t + k%41] for k < 123 and the
            # [32,1,1,1,1] norm-carrier rows below (all host-baked)
            wt = [
                wpool.tile([128, NCHUNK, T], F8, tag=f"w{b}", name=f"wt{b}")
                for b in range(BC)
            ]
            # contiguous head tile for batch 0's first 256 window columns
            wh = wpool.tile([128, NCHUNK, 256], F8, tag="wh", name="wh")

            # DMA choreography: everything on the gpsimd SWDGE queue (spreads
            # packets over all 16 SDMA engines; FIFO order = landing order).
            # The first matmul's exact operand regions (chunk-pair 0 of the
            # codebook halves + head windows) land first; the subtile
            # processing order below consumes column blocks in landing
            # order, giving every later transfer a wide deadline.
            nc.gpsimd.dma_start(cbt[0][:], cb_d[0])
            nc.gpsimd.dma_start(wh[:], xh_d[:])
            nc.gpsimd.dma_start(cbt[1][:], cb_d[1])
            nc.gpsimd.dma_start(wt[0][:, :, 256:2048], xm_d[:])
            nc.gpsimd.dma_start(wt[1][:, :, 0:2048], x1_d[:, :, 0:2048])
            nc.gpsimd.dma_start(wt[0][:, :, 2048:4096], xg_d[:])
            nc.gpsimd.dma_start(wt[1][:, :, 2048:4096], x1_d[:, :, 2048:4096])

            warm_src = misc.tile([128, 128], BF)
            nc.vector.memset(warm_src[:], 0.5)

            # HAM warmup: PE busy early so the clock is 2.4 GHz when the
            # real matmuls start.  Short N=128 matmuls so a late operand
            # arrival is never stuck behind a long warmup op.
            with tc.tile_pool(name="pwarm", bufs=1, space="PSUM") as pwarm:
                wps = pwarm.tile([128, 128], F32)
                for _ in range(NWARM):
                    nc.tensor.matmul(
                        wps[:], warm_src[:], warm_src[:], start=True, stop=True
                    )

            # main loop: 32 pairs of 128-window subtiles.  Drain pipeline per
            # pair (PE pace ~1.73us/pair): the two PSUM readers run fully in
            # parallel — ScalarE converts the h1 half [512:1024) to fp16
            # (~1.1us) while VectorE folds the h0 half against itself
            # straight out of PSUM (~1.2us) — so the PSUM banks free ~1.3us
            # after the pair's last matmul, well inside the two-pair slack.
            # VectorE then folds the fp16 half (~0.43us) and the pair's
            # 512-wide partial minima ship to DRAM; the host finishes.
            with (
                tc.tile_pool(name="pmain", bufs=2, space="PSUM") as pmain,
                tc.tile_pool(name="cvt", bufs=4) as cvt,
            ):
                # subtile order matches the DMA landing order: batch 0 cols
                # [0:2048), batch 1 [0:2048), batch 0 [2048:), batch 1
                # [2048:).  The host sum is order-invariant.
                sub_order = (
                    [(0, t) for t in range(16)]
                    + [(1, t) for t in range(16)]
                    + [(0, t) for t in range(16, 32)]
                    + [(1, t) for t in range(16, 32)]
                )
                prev = None
                for pair in range(NSUB // 2):
                    # separate PSUM tiles per code-half: each matmul region
                    # then has exactly ONE drain reader (h0 -> the VectorE
                    # reduce, h1 -> the ScalarE convert), so the WAR wait of
                    # the pair-after-next's matmuls is a single fast
                    # semaphore instead of a coalesced chain.
                    ps0 = pmain.tile([128, 2, 512], F32, tag="ph0", name=f"ps0_{pair}")
                    ps1 = pmain.tile([128, 2, 512], F32, tag="ph1", name=f"ps1_{pair}")
                    for s in range(2):
                        b, tix = sub_order[pair * 2 + s]
                        toff = tix * 128
                        if b == 0 and toff < 256:
                            stat_t, soff = wh, toff
                        else:
                            stat_t, soff = wt[b], toff
                        # jp-outer: each stationary window block feeds both
                        # code-half matmuls before the next weight load
                        for jp in (0, 2):
                            stat = stat_t[:, jp : jp + 2, soff : soff + 128]
                            for h, pst in ((0, ps0), (1, ps1)):
                                nc.tensor.matmul(
                                    pst[:, s, :],
                                    stat,
                                    cbt[h][:, jp : jp + 2, :],
                                    start=(jp == 0),
                                    stop=(jp == 2),
                                    perf_mode=DR,
                                )
                    hi16 = cvt.tile([128, 2, 512], F16, tag="hi16")
                    jout = cvt.tile([128, 2, 257], F16, tag="jout")
                    nc.scalar.activation(hi16[:], ps1[:], ACT.Copy)
                    # the PSUM-freeing reduce is emitted ahead of the
                    # PREVIOUS pair's fp16 fold, so it sits at the front of
                    # VectorE's queue the moment its PSUM data is ready
                    nc.vector.tensor_reduce(
                        jout[:, :, 256:257],
                        ps0[:],
                        axis=mybir.AxisListType.X,
                        op=OP.min,
                    )
                    if prev is not None:
                        phi, pjout, ppair = prev
                        nc.vector.tensor_tensor(
                            pjout[:, :, 0:256],
                            phi[:, :, 0:256],
                            phi[:, :, 256:512],
                            op=OP.min,
                        )
                        # ship on the idle sync ring so the SWDGE queue
                        # stays dedicated to the input stream
                        nc.sync.dma_start(outw_d[ppair], pjout[:])
                    prev = (hi16, jout, pair)
                # flush the deferred fold + out of the final pair
                phi, pjout, ppair = prev
                nc.vector.tensor_tensor(
                    pjout[:, :, 0:256],
                    phi[:, :, 0:256],
                    phi[:, :, 256:512],
                    op=OP.min,
                )
                nc.sync.dma_start(outw_d[ppair], pjout[:])

    nc.compile()
    return nc


def get_nc():
    if "nc" not in _CACHED:
        _CACHED["nc"] = _build_nc()
    return _CACHED["nc"]


def _host_prep(x, codebook):
    """Pad/cast/shard the inputs; returns (per-core in_maps, self term)."""
    x = np.asarray(x, dtype=np.float32)
    codebook = np.asarray(codebook, dtype=np.float32)

    x8 = x.astype(FP8NP)
    xw = np.zeros((B, P, TP), dtype=FP8NP)
    xw[:, :, PAD : PAD + T] = x8

    # value of the fp8-rounded codebook, exactly scaled by -2
    cbb = codebook.astype(FP8NP).astype(np.float32)
    rhs = np.zeros((128, NCHUNK, K), dtype=np.float32)
    for c in range(NCHUNK):
        rhs[:CHROWS, c, :] = -2.0 * cbb[:, CHROWS * c : CHROWS * (c + 1)].T
    rhs8 = rhs.astype(FP8NP)

    # ||e||^2 rows: c = 32*r0 + r1 + r2 in fp8, paired with the [32,1,1,1,1]
    # norm-carrier rows of the window tiles
    cnorm = (cbb.astype(np.float64) ** 2).sum(axis=1).astype(np.float32)
    r0 = (cnorm / 32.0).astype(FP8NP)
    rem1 = cnorm - 32.0 * r0.astype(np.float32)
    r1 = rem1.astype(FP8NP)
    rem2 = rem1 - r1.astype(np.float32)
    r2 = rem2.astype(FP8NP)
    rhs8[CHROWS, 0, :] = r0
    rhs8[CHROWS + 1, 0, :] = r1
    rhs8[CHROWS + 2, 0, :] = r2
    # code-half major layout: [h, 128, chunk, 512]
    cb_h = np.ascontiguousarray(
        np.stack([rhs8[:, :, 0:512], rhs8[:, :, 512:1024]])
    )

    # dense im2col expansion of every batch: xe[b][k, c, t] =
    # xw[b, 3c + k//41, t + k%41] for k < 123; norm-carrier rows below.
    # Built from a zero-copy sliding-window view + one big transpose.
    sw = np.lib.stride_tricks.sliding_window_view(xw, WIN, axis=2)  # [B,P,T+1? ,41]
    sw = sw[:, :, :T, :]                                            # [B, P, T, 41]
    xe_all = np.empty((B, 128, NCHUNK, T), dtype=FP8NP)
    # p = 3c + j  ->  row k = 41j + tap, chunk c
    arr = sw.reshape(B, NCHUNK, 3, T, WIN).transpose(0, 2, 4, 1, 3)  # [B,3,41,c,T]
    xe_all[:, :CHROWS] = arr.reshape(B, CHROWS, NCHUNK, T)
    xe_all[:, CHROWS:] = 1.0
    xe_all[:, CHROWS] = FP8NP(32.0)

    # host-side ||f||^2 term via the window-count trick
    tau = np.arange(TP, dtype=np.float64)
    cnt = np.minimum(np.minimum(tau + 1.0, float(WIN)), float(TP) - tau)
    xf = xw.astype(np.float64)
    self_term = float((xf * xf * cnt[None, None, :]).sum())

    in_maps = []
    for i in range(NCORES):
        xe0 = xe_all[BC * i]
        xe1 = xe_all[BC * i + 1]
        in_maps.append(
            {
                "xh": np.ascontiguousarray(xe0[:, :, 0:256]),
                "xm": np.ascontiguousarray(xe0[:, :, 256:2048]),
                "xg": np.ascontiguousarray(xe0[:, :, 2048:4096]),
                "x1": xe1,
                "cb": cb_h,
            }
        )
    return in_maps, self_term


def kernel(x, codebook):
    from concourse.bass_utils import run_bass_kernel_spmd

    nc = get_nc()
    in_maps, self_term = _host_prep(x, codebook)
    res = run_bass_kernel_spmd(nc, in_maps, core_ids=list(range(NCORES)))
    total = np.float64(self_term)
    for r in res.results:
        # [pair, window, subtile, 513] fp16 partials -> per-window minima
        mins = r["outw"].astype(np.float32).min(axis=-1)
        total += mins.astype(np.float64).sum()
    return np.array(np.float32(SCALE * total))


# revision 70
# speedup vs baseline: 1.0417x; 1.0417x over previous
"""Trainium2 Bass kernel for the VQ commitment-loss problem (fp8 DoubleRow).

Math
----
reference loss = 0.25 * mean((codebook[argmin_k dist] - flat)**2)
               = 0.25/(B*T*D) * sum_n min_k ||flat_n - e_k||^2
since the gathered quantized row realizes exactly the min squared distance.

min_k ||f - e||^2 = ||f||^2 + min_k (||e_k||^2 - 2 f.e_k)

The ||f||^2 term is a tiny O(B*P*T) reduction of the (fp8-rounded) input,
computed on the host via the window-count trick.  The device computes only
the dominant O(N*K*D) term: per core (2 of 16 batches)

  - sum_n min_k (||e_k||^2 - 2 f_n.e_k) via fp8e4 DoubleRow TensorE matmuls
    (256-deep contraction per pass): window tiles [128, 4sub, T] are the
    stationary operand, the codebook scaled by -2 is the moving operand
    (two [128, 4sub, 512] code-half tiles).  ||e_k||^2 rides as three extra
    contraction rows (32*r0 + r1 + r2 fp8 decomposition, precomputed on the
    host, paired with a [32,1,1,1,1] column in the window operand).
  - each code-half accumulates into its OWN 2-bank PSUM tile so every
    matmul region has exactly one drain reader and the write-after-read
    wait of the pair-after-next's matmuls is a single fast semaphore
    (engines can wait on only one semaphore per instruction, so shared
    tiles force Tile to serialize the whole drain into the PE path).
  - drain per pair (PE pace ~1.73us): ScalarE converts the h1 half to
    fp16 (~1.11us) in parallel with a VectorE free-axis min-reduce of
    the h0 half straight out of PSUM (~1.23us); VectorE then folds the
    fp16 half once (~0.42us) and the 257-wide partials ship to DRAM on
    the otherwise-idle sync HWDGE ring.  The host finishes the last min
    levels and the sum (order-invariant).

DMA: the host pre-expands the ENTIRE im2col window tensor (dense,
contiguous per-partition lines) so the device needs only a handful of
large transfers on the gpsimd SWDGE queue (16 SDMA engines) instead of
descriptor-heavy strided expansion; the subtile processing order
(b0[0:2048), b1[0:2048), b0 tail, b1 tail) matches the FIFO landing
order so every transfer has a wide deadline.  N=128 warmup matmuls
bridge the PE HAM clock to 2.4 GHz until the first operands land.

Host side pads/casts/shards inputs, precomputes the codebook norm rows,
the dense window tensor and the ||f||^2 self term, and reduces the
per-core partials.
"""

import numpy as np
import ml_dtypes

B, P, T = 16, 12, 4096
WIN = 41
PAD = (WIN - 1) // 2          # 20
K = 1024
D = P * WIN                   # 492
COMMITMENT_COST = 0.25

NCORES = 8
BC = B // NCORES              # batches per core = 2
TP = T + 2 * PAD              # padded time = 4136
NCHUNK = 4                    # contraction subtiles: 3 pellets * 41 taps = 123 rows
CHROWS = 3 * WIN              # 123
NSUB = BC * T // 128          # 64 subtiles of 128 windows per core
NWARM = 34                    # HAM warmup matmuls (bridge PE to main-loop start)

SCALE = COMMITMENT_COST / (B * T * D)

FP8NP = ml_dtypes.float8_e4m3

_CACHED = {}


def _build_nc():
    import concourse.bacc as bacc
    import concourse.bass as bass
    import concourse.mybir as mybir
    import concourse.tile as tile

    BF = mybir.dt.bfloat16
    F32 = mybir.dt.float32
    F16 = mybir.dt.float16
    F8 = mybir.dt.float8e4
    OP = mybir.AluOpType
    ACT = mybir.ActivationFunctionType
    DR = mybir.MatmulPerfMode.DoubleRow

    nc = bacc.Bacc("TRN2", target_bir_lowering=False, debug=False)

    # host pre-expanded window tensors for this core's two batches; the head
    # columns live in their own tensor so the first DMA has a fully
    # contiguous destination tile (large descriptors, fast landing)
    xh_d = nc.dram_tensor("xh", [128, NCHUNK, 256], F8, kind="ExternalInput")
    xm_d = nc.dram_tensor("xm", [128, NCHUNK, 1792], F8, kind="ExternalInput")
    xg_d = nc.dram_tensor("xg", [128, NCHUNK, 2048], F8, kind="ExternalInput")
    x1_d = nc.dram_tensor("x1", [128, NCHUNK, T], F8, kind="ExternalInput")
    cb_d = nc.dram_tensor("cb", [2, 128, NCHUNK, 512], F8, kind="ExternalInput")
    # per-pair partials [pair, window, subtile, 257]: cols 0:256 are the
    # once-folded fp16 h1 distances, col 256 is the h0 min; host finishes
    # the reduction + the sum
    outw_d = nc.dram_tensor("outw", [NSUB // 2, 128, 2, 257], F16, kind="ExternalOutput")

    with tile.TileContext(nc) as tc:
        with (
            tc.tile_pool(name="cbpool", bufs=1) as cbpool,
            tc.tile_pool(name="wpool", bufs=1) as wpool,
            tc.tile_pool(name="misc", bufs=1) as misc,
        ):
            # resident codebook code-half tiles [k, chunk, code]; rows 123..125
            # of chunk 0 carry the host ||e||^2 fp8 decomposition
            cbt = [
                cbpool.tile([128, NCHUNK, 512], F8, tag=f"cb{h}", name=f"cbt{h}")
                for h in range(2)
            ]
            # resident window tiles wt[b]: [128, chunk, T] fp8 with
            # wt[b][k, c, t] = xw[b, 3c + k//41, t + k%41] for k < 123 and the
            # [32,1,1,1,1] norm-carrier rows below (all host-baked)
            wt = [
                wpool.tile([128, NCHUNK, T], F8, tag=f"w{b}", name=f"wt{b}")
                for b in range(BC)
            ]
            # contiguous head tile for batch 0's first 256 window columns
            wh = wpool.tile([128, NCHUNK, 256], F8, tag="wh", name="wh")

            # DMA choreography: everything on the gpsimd SWDGE queue (spreads
            # packets over all 16 SDMA engines; FIFO order = landing order).
            # The first matmul's exact operand regions (chunk-pair 0 of the
            # codebook halves + head windows) land first; the subtile
            # processing order below consumes column blocks in landing
            # order, giving every later transfer a wide deadline.
            nc.gpsimd.dma_start(cbt[0][:], cb_d[0])
            nc.gpsimd.dma_start(wh[:], xh_d[:])
            nc.gpsimd.dma_start(cbt[1][:], cb_d[1])
            nc.gpsimd.dma_start(wt[0][:, :, 256:2048], xm_d[:])
            nc.gpsimd.dma_start(wt[1][:, :, 0:2048], x1_d[:, :, 0:2048])
            nc.gpsimd.dma_start(wt[0][:, :, 2048:4096], xg_d[:])
            nc.gpsimd.dma_start(wt[1][:, :, 2048:4096], x1_d[:, :, 2048:4096])

            warm_src = misc.tile([128, 128], BF)
            nc.vector.memset(warm_src[:], 0.5)

            # HAM warmup: PE busy early so the clock is 2.4 GHz when the
            # real matmuls start.  Short N=128 matmuls so a late operand
            # arrival is never stuck behind a long warmup op.
            with tc.tile_pool(name="pwarm", bufs=1, space="PSUM") as pwarm:
                wps = pwarm.tile([128, 128], F32)
                for _ in range(NWARM):
                    nc.tensor.matmul(
                        wps[:], warm_src[:], warm_src[:], start=True, stop=True
                    )

            # main loop: 32 pairs of 128-window subtiles.  Drain pipeline per
            # pair (PE pace ~1.73us/pair): the two PSUM readers run fully in
            # parallel — ScalarE converts the h1 half [512:1024) to fp16
            # (~1.1us) while VectorE folds the h0 half against itself
            # straight out of PSUM (~1.2us) — so the PSUM banks free ~1.3us
            # after the pair's last matmul, well inside the two-pair slack.
            # VectorE then folds the fp16 half (~0.43us) and the pair's
            # 512-wide partial minima ship to DRAM; the host finishes.
            with (
                tc.tile_pool(name="pmain", bufs=2, space="PSUM") as pmain,
                tc.tile_pool(name="cvt", bufs=4) as cvt,
            ):
                # subtile order matches the DMA landing order: batch 0 cols
                # [0:2048), batch 1 [0:2048), batch 0 [2048:), batch 1
                # [2048:).  The host sum is order-invariant.
                sub_order = (
                    [(0, t) for t in range(16)]
                    + [(1, t) for t in range(16)]
                    + [(0, t) for t in range(16, 32)]
                    + [(1, t) for t in range(16, 32)]
                )
                for pair in range(NSUB // 2):
                    # separate PSUM tiles per code-half: each matmul region
                    # then has exactly ONE drain reader (h0 -> the VectorE
                    # reduce, h1 -> the ScalarE convert), so the WAR wait of
                    # the pair-after-next's matmuls is a single fast
                    # semaphore instead of a coalesced chain.
                    ps0 = pmain.tile([128, 2, 512], F32, tag="ph0", name=f"ps0_{pair}")
                    ps1 = pmain.tile([128, 2, 512], F32, tag="ph1", name=f"ps1_{pair}")
                    for s in range(2):
                        b, tix = sub_order[pair * 2 + s]
                        toff = tix * 128
                        if b == 0 and toff < 256:
                            stat_t, soff = wh, toff
                        else:
                            stat_t, soff = wt[b], toff
                        # jp-outer: each stationary window block feeds both
                        # code-half matmuls before the next weight load
                        for jp in (0, 2):
                            stat = stat_t[:, jp : jp + 2, soff : soff + 128]
                            for h, pst in ((0, ps0), (1, ps1)):
                                nc.tensor.matmul(
                                    pst[:, s, :],
                                    stat,
                                    cbt[h][:, jp : jp + 2, :],
                                    start=(jp == 0),
                                    stop=(jp == 2),
                                    perf_mode=DR,
                                )
                    hi16 = cvt.tile([128, 2, 512], F16, tag="hi16")
                    jout = cvt.tile([128, 2, 257], F16, tag="jout")
                    nc.scalar.activation(hi16[:], ps1[:], ACT.Copy)
                    nc.vector.tensor_reduce(
                        jout[:, :, 256:257],
                        ps0[:],
                        axis=mybir.AxisListType.X,
                        op=OP.min,
                    )
                    nc.vector.tensor_tensor(
                        jout[:, :, 0:256], hi16[:, :, 0:256], hi16[:, :, 256:512],
                        op=OP.min,
                    )
                    # ship the pair's partials on the idle sync ring so the
                    # SWDGE queue stays dedicated to the input stream
                    nc.sync.dma_start(outw_d[pair], jout[:])

    nc.compile()
    return nc


def get_nc():
    if "nc" not in _CACHED:
        _CACHED["nc"] = _build_nc()
    return _CACHED["nc"]


def _host_prep(x, codebook):
    """Pad/cast/shard the inputs; returns (per-core in_maps, self term)."""
    x = np.asarray(x, dtype=np.float32)
    codebook = np.asarray(codebook, dtype=np.float32)

    x8 = x.astype(FP8NP)
    xw = np.zeros((B, P, TP), dtype=FP8NP)
    xw[:, :, PAD : PAD + T] = x8

    # value of the fp8-rounded codebook, exactly scaled by -2
    cbb = codebook.astype(FP8NP).astype(np.float32)
    rhs = np.zeros((128, NCHUNK, K), dtype=np.float32)
    for c in range(NCHUNK):
        rhs[:CHROWS, c, :] = -2.0 * cbb[:, CHROWS * c : CHROWS * (c + 1)].T
    rhs8 = rhs.astype(FP8NP)

    # ||e||^2 rows: c = 32*r0 + r1 + r2 in fp8, paired with the [32,1,1,1,1]
    # norm-carrier rows of the window tiles
    cnorm = (cbb.astype(np.float64) ** 2).sum(axis=1).astype(np.float32)
    r0 = (cnorm / 32.0).astype(FP8NP)
    rem1 = cnorm - 32.0 * r0.astype(np.float32)
    r1 = rem1.astype(FP8NP)
    rem2 = rem1 - r1.astype(np.float32)
    r2 = rem2.astype(FP8NP)
    rhs8[CHROWS, 0, :] = r0
    rhs8[CHROWS + 1, 0, :] = r1
    rhs8[CHROWS + 2, 0, :] = r2
    # code-half major layout: [h, 128, chunk, 512]
    cb_h = np.ascontiguousarray(
        np.stack([rhs8[:, :, 0:512], rhs8[:, :, 512:1024]])
    )

    # dense im2col expansion of every batch: xe[b][k, c, t] =
    # xw[b, 3c + k//41, t + k%41] for k < 123; norm-carrier rows below.
    # Built from a zero-copy sliding-window view + one big transpose.
    sw = np.lib.stride_tricks.sliding_window_view(xw, WIN, axis=2)  # [B,P,T+1? ,41]
    sw = sw[:, :, :T, :]                                            # [B, P, T, 41]
    xe_all = np.empty((B, 128, NCHUNK, T), dtype=FP8NP)
    # p = 3c + j  ->  row k = 41j + tap, chunk c
    arr = sw.reshape(B, NCHUNK, 3, T, WIN).transpose(0, 2, 4, 1, 3)  # [B,3,41,c,T]
    xe_all[:, :CHROWS] = arr.reshape(B, CHROWS, NCHUNK, T)
    xe_all[:, CHROWS:] = 1.0
    xe_all[:, CHROWS] = FP8NP(32.0)

    # host-side ||f||^2 term via the window-count trick
    tau = np.arange(TP, dtype=np.float64)
    cnt = np.minimum(np.minimum(tau + 1.0, float(WIN)), float(TP) - tau)
    xf = xw.astype(np.float64)
    self_term = float((xf * xf * cnt[None, None, :]).sum())

    in_maps = []
    for i in range(NCORES):
        xe0 = xe_all[BC * i]
        xe1 = xe_all[BC * i + 1]
        in_maps.append(
            {
                "xh": np.ascontiguousarray(xe0[:, :, 0:256]),
                "xm": np.ascontiguousarray(xe0[:, :, 256:2048]),
                "xg": np.ascontiguousarray(xe0[:, :, 2048:4096]),
                "x1": xe1,
                "cb": cb_h,
            }
        )
    return in_maps, self_term


def kernel(x, codebook):
    from concourse.bass_utils import run_bass_kernel_spmd

    nc = get_nc()
    in_maps, self_term = _host_prep(x, codebook)
    res = run_bass_kernel_spmd(nc, in_maps, core_ids=list(range(NCORES)))
    total = np.float64(self_term)
    for r in res.results:
        # [pair, window, subtile, 513] fp16 partials -> per-window minima
        mins = r["outw"].astype(np.float32).min(axis=-1)
        total += mins.astype(np.float64).sum()
    return np.array(np.float32(SCALE * total))


# revision 75
# speedup vs baseline: 1.0723x; 1.0293x over previous
"""Trainium2 Bass kernel for the VQ commitment-loss problem (fp8 DoubleRow).

Math
----
reference loss = 0.25 * mean((codebook[argmin_k dist] - flat)**2)
               = 0.25/(B*T*D) * sum_n min_k ||flat_n - e_k||^2
since the gathered quantized row realizes exactly the min squared distance.

min_k ||f - e||^2 = ||f||^2 + min_k (||e_k||^2 - 2 f.e_k)

The ||f||^2 term is a tiny O(B*P*T) reduction of the (fp8-rounded) input,
computed on the host via the window-count trick.  The device computes only
the dominant O(N*K*D) term: per core (2 of 16 batches)

  - sum_n min_k (||e_k||^2 - 2 f_n.e_k) via fp8e4 DoubleRow TensorE matmuls
    (256-deep contraction per pass): window tiles [128, 4sub, T] are the
    stationary operand, the codebook scaled by -2 is the moving operand
    (two [128, 4sub, 512] code-half tiles).  ||e_k||^2 rides as three extra
    contraction rows (32*r0 + r1 + r2 fp8 decomposition, precomputed on the
    host, paired with a [32,1,1,1,1] column in the window operand).
  - each code-half accumulates into its OWN 2-bank PSUM tile so every
    matmul region has exactly one drain reader and the write-after-read
    wait of the pair-after-next's matmuls is a single fast semaphore
    (engines can wait on only one semaphore per instruction, so shared
    tiles force Tile to serialize the whole drain into the PE path).
  - drain per pair (PE pace ~1.73us): ScalarE converts the h1 half to
    fp16 (~1.11us) in parallel with a VectorE free-axis min-reduce of
    the h0 half straight out of PSUM (~1.23us); VectorE then folds the
    fp16 half once (~0.42us) and the 257-wide partials ship to DRAM on
    the otherwise-idle sync HWDGE ring.  The host finishes the last min
    levels and the sum (order-invariant).

DMA: the host pre-expands the ENTIRE im2col window tensor (dense,
contiguous per-partition lines) so the device needs only a handful of
large transfers on the gpsimd SWDGE queue (16 SDMA engines) instead of
descriptor-heavy strided expansion; the subtile processing order
(b0[0:2048), b1[0:2048), b0 tail, b1 tail) matches the FIFO landing
order so every transfer has a wide deadline.  N=128 warmup matmuls
bridge the PE HAM clock to 2.4 GHz until the first operands land.

Host side pads/casts/shards inputs, precomputes the codebook norm rows,
the dense window tensor and the ||f||^2 self term, and reduces the
per-core partials.
"""

import numpy as np
import ml_dtypes

B, P, T = 16, 12, 4096
WIN = 41
PAD = (WIN - 1) // 2          # 20
K = 1024
D = P * WIN                   # 492
COMMITMENT_COST = 0.25

NCORES = 8
BC = B // NCORES              # batches per core = 2
TP = T + 2 * PAD              # padded time = 4136
NCHUNK = 4                    # contraction subtiles: 3 pellets * 41 taps = 123 rows
CHROWS = 3 * WIN              # 123
NSUB = BC * T // 128          # 64 subtiles of 128 windows per core
NWARM = 34                    # HAM warmup matmuls (bridge PE to main-loop start)

SCALE = COMMITMENT_COST / (B * T * D)

FP8NP = ml_dtypes.float8_e4m3

_CACHED = {}


def _build_nc():
    import concourse.bacc as bacc
    import concourse.bass as bass
    import concourse.mybir as mybir
    import concourse.tile as tile

    BF = mybir.dt.bfloat16
    F32 = mybir.dt.float32
    F16 = mybir.dt.float16
    F8 = mybir.dt.float8e4
    OP = mybir.AluOpType
    ACT = mybir.ActivationFunctionType
    DR = mybir.MatmulPerfMode.DoubleRow

    nc = bacc.Bacc("TRN2", target_bir_lowering=False, debug=False)

    # host pre-expanded window tensors for this core's two batches; the head
    # columns live in their own tensor so the first DMA has a fully
    # contiguous destination tile (large descriptors, fast landing)
    xh_d = nc.dram_tensor("xh", [128, NCHUNK, 1024], F8, kind="ExternalInput")
    xm_d = nc.dram_tensor("xm", [128, NCHUNK, 1024], F8, kind="ExternalInput")
    xg_d = nc.dram_tensor("xg", [128, NCHUNK, 2048], F8, kind="ExternalInput")
    x1_d = nc.dram_tensor("x1", [128, NCHUNK, T], F8, kind="ExternalInput")
    cb_d = nc.dram_tensor("cb", [2, 128, NCHUNK, 512], F8, kind="ExternalInput")
    # per-pair partials [pair, window, subtile, 257]: cols 0:256 are the
    # once-folded fp16 h1 distances, col 256 is the h0 min; host finishes
    # the reduction + the sum
    outw_d = nc.dram_tensor("outw", [NSUB // 2, 128, 2, 257], F16, kind="ExternalOutput")

    with tile.TileContext(nc) as tc:
        with (
            tc.tile_pool(name="cbpool", bufs=1) as cbpool,
            tc.tile_pool(name="wpool", bufs=1) as wpool,
            tc.tile_pool(name="misc", bufs=1) as misc,
        ):
            # resident codebook code-half tiles [k, chunk, code]; rows 123..125
            # of chunk 0 carry the host ||e||^2 fp8 decomposition
            cbt = [
                cbpool.tile([128, NCHUNK, 512], F8, tag=f"cb{h}", name=f"cbt{h}")
                for h in range(2)
            ]
            # resident window tiles wt[b]: [128, chunk, T] fp8 with
            # wt[b][k, c, t] = xw[b, 3c + k//41, t + k%41] for k < 123 and the
            # [32,1,1,1,1] norm-carrier rows below (all host-baked)
            wt = [
                wpool.tile([128, NCHUNK, T], F8, tag=f"w{b}", name=f"wt{b}")
                for b in range(BC)
            ]
            # contiguous head tile for batch 0's first 1024 window columns
            # (8 subtiles): one early-landing transfer covers the whole
            # main-loop ramp, so no early subtile is gated on the bulk
            # window stream completing
            wh = wpool.tile([128, NCHUNK, 1024], F8, tag="wh", name="wh")

            # DMA choreography: everything on the gpsimd SWDGE queue (spreads
            # packets over all 16 SDMA engines; FIFO order = landing order).
            # The first matmul's exact operand regions (chunk-pair 0 of the
            # codebook halves + head windows) land first; the subtile
            # processing order below consumes column blocks in landing
            # order, giving every later transfer a wide deadline.
            nc.gpsimd.dma_start(cbt[0][:], cb_d[0])
            nc.gpsimd.dma_start(wh[:], xh_d[:])
            nc.gpsimd.dma_start(cbt[1][:], cb_d[1])
            nc.gpsimd.dma_start(wt[0][:, :, 1024:2048], xm_d[:])
            nc.gpsimd.dma_start(wt[1][:, :, 0:2048], x1_d[:, :, 0:2048])
            nc.gpsimd.dma_start(wt[0][:, :, 2048:4096], xg_d[:])
            nc.gpsimd.dma_start(wt[1][:, :, 2048:4096], x1_d[:, :, 2048:4096])

            warm_src = misc.tile([128, 128], BF)
            nc.vector.memset(warm_src[:], 0.5)

            # HAM warmup: PE busy early so the clock is 2.4 GHz when the
            # real matmuls start.  Short N=128 matmuls so a late operand
            # arrival is never stuck behind a long warmup op.
            with tc.tile_pool(name="pwarm", bufs=1, space="PSUM") as pwarm:
                wps = pwarm.tile([128, 128], F32)
                for _ in range(NWARM):
                    nc.tensor.matmul(
                        wps[:], warm_src[:], warm_src[:], start=True, stop=True
                    )

            # main loop: 32 pairs of 128-window subtiles.  Drain pipeline per
            # pair (PE pace ~1.73us/pair): the two PSUM readers run fully in
            # parallel — ScalarE converts the h1 half [512:1024) to fp16
            # (~1.1us) while VectorE folds the h0 half against itself
            # straight out of PSUM (~1.2us) — so the PSUM banks free ~1.3us
            # after the pair's last matmul, well inside the two-pair slack.
            # VectorE then folds the fp16 half (~0.43us) and the pair's
            # 512-wide partial minima ship to DRAM; the host finishes.
            with (
                tc.tile_pool(name="pmain", bufs=2, space="PSUM") as pmain,
                tc.tile_pool(name="cvt", bufs=4) as cvt,
            ):
                # subtile order matches the DMA landing order: batch 0 cols
                # [0:2048), batch 1 [0:2048), batch 0 [2048:), batch 1
                # [2048:).  The host sum is order-invariant.
                sub_order = (
                    [(0, t) for t in range(16)]
                    + [(1, t) for t in range(16)]
                    + [(0, t) for t in range(16, 32)]
                    + [(1, t) for t in range(16, 32)]
                )
                for pair in range(NSUB // 2):
                    # separate PSUM tiles per code-half: each matmul region
                    # then has exactly ONE drain reader (h0 -> the VectorE
                    # reduce, h1 -> the ScalarE convert), so the WAR wait of
                    # the pair-after-next's matmuls is a single fast
                    # semaphore instead of a coalesced chain.
                    ps0 = pmain.tile([128, 2, 512], F32, tag="ph0", name=f"ps0_{pair}")
                    ps1 = pmain.tile([128, 2, 512], F32, tag="ph1", name=f"ps1_{pair}")
                    for s in range(2):
                        b, tix = sub_order[pair * 2 + s]
                        toff = tix * 128
                        if b == 0 and toff < 1024:
                            stat_t, soff = wh, toff
                        else:
                            stat_t, soff = wt[b], toff
                        # jp-outer: each stationary window block feeds both
                        # code-half matmuls before the next weight load
                        for jp in (0, 2):
                            stat = stat_t[:, jp : jp + 2, soff : soff + 128]
                            for h, pst in ((0, ps0), (1, ps1)):
                                nc.tensor.matmul(
                                    pst[:, s, :],
                                    stat,
                                    cbt[h][:, jp : jp + 2, :],
                                    start=(jp == 0),
                                    stop=(jp == 2),
                                    perf_mode=DR,
                                )
                    hi16 = cvt.tile([128, 2, 512], F16, tag="hi16")
                    jout = cvt.tile([128, 2, 257], F16, tag="jout")
                    nc.scalar.activation(hi16[:], ps1[:], ACT.Copy)
                    nc.vector.tensor_reduce(
                        jout[:, :, 256:257],
                        ps0[:],
                        axis=mybir.AxisListType.X,
                        op=OP.min,
                    )
                    nc.vector.tensor_tensor(
                        jout[:, :, 0:256], hi16[:, :, 0:256], hi16[:, :, 256:512],
                        op=OP.min,
                    )
                    # ship the pair's partials on the idle sync ring so the
                    # SWDGE queue stays dedicated to the input stream
                    nc.sync.dma_start(outw_d[pair], jout[:])

    nc.compile()
    return nc


def get_nc():
    if "nc" not in _CACHED:
        _CACHED["nc"] = _build_nc()
    return _CACHED["nc"]


def _host_prep(x, codebook):
    """Pad/cast/shard the inputs; returns (per-core in_maps, self term)."""
    x = np.asarray(x, dtype=np.float32)
    codebook = np.asarray(codebook, dtype=np.float32)

    x8 = x.astype(FP8NP)
    xw = np.zeros((B, P, TP), dtype=FP8NP)
    xw[:, :, PAD : PAD + T] = x8

    # value of the fp8-rounded codebook, exactly scaled by -2
    cbb = codebook.astype(FP8NP).astype(np.float32)
    rhs = np.zeros((128, NCHUNK, K), dtype=np.float32)
    for c in range(NCHUNK):
        rhs[:CHROWS, c, :] = -2.0 * cbb[:, CHROWS * c : CHROWS * (c + 1)].T
    rhs8 = rhs.astype(FP8NP)

    # ||e||^2 rows: c = 32*r0 + r1 + r2 in fp8, paired with the [32,1,1,1,1]
    # norm-carrier rows of the window tiles
    cnorm = (cbb.astype(np.float64) ** 2).sum(axis=1).astype(np.float32)
    r0 = (cnorm / 32.0).astype(FP8NP)
    rem1 = cnorm - 32.0 * r0.astype(np.float32)
    r1 = rem1.astype(FP8NP)
    rem2 = rem1 - r1.astype(np.float32)
    r2 = rem2.astype(FP8NP)
    rhs8[CHROWS, 0, :] = r0
    rhs8[CHROWS + 1, 0, :] = r1
    rhs8[CHROWS + 2, 0, :] = r2
    # code-half major layout: [h, 128, chunk, 512]
    cb_h = np.ascontiguousarray(
        np.stack([rhs8[:, :, 0:512], rhs8[:, :, 512:1024]])
    )

    # dense im2col expansion of every batch: xe[b][k, c, t] =
    # xw[b, 3c + k//41, t + k%41] for k < 123; norm-carrier rows below.
    # Built from a zero-copy sliding-window view + one big transpose.
    sw = np.lib.stride_tricks.sliding_window_view(xw, WIN, axis=2)  # [B,P,T+1? ,41]
    sw = sw[:, :, :T, :]                                            # [B, P, T, 41]
    xe_all = np.empty((B, 128, NCHUNK, T), dtype=FP8NP)
    # p = 3c + j  ->  row k = 41j + tap, chunk c
    arr = sw.reshape(B, NCHUNK, 3, T, WIN).transpose(0, 2, 4, 1, 3)  # [B,3,41,c,T]
    xe_all[:, :CHROWS] = arr.reshape(B, CHROWS, NCHUNK, T)
    xe_all[:, CHROWS:] = 1.0
    xe_all[:, CHROWS] = FP8NP(32.0)

    # host-side ||f||^2 term via the window-count trick
    tau = np.arange(TP, dtype=np.float64)
    cnt = np.minimum(np.minimum(tau + 1.0, float(WIN)), float(TP) - tau)
    xf = xw.astype(np.float64)
    self_term = float((xf * xf * cnt[None, None, :]).sum())

    in_maps = []
    for i in range(NCORES):
        xe0 = xe_all[BC * i]
        xe1 = xe_all[BC * i + 1]
        in_maps.append(
            {
                "xh": np.ascontiguousarray(xe0[:, :, 0:1024]),
                "xm": np.ascontiguousarray(xe0[:, :, 1024:2048]),
                "xg": np.ascontiguousarray(xe0[:, :, 2048:4096]),
                "x1": xe1,
                "cb": cb_h,
            }
        )
    return in_maps, self_term


def kernel(x, codebook):
    from concourse.bass_utils import run_bass_kernel_spmd

    nc = get_nc()
    in_maps, self_term = _host_prep(x, codebook)
    res = run_bass_kernel_spmd(nc, in_maps, core_ids=list(range(NCORES)))
    total = np.float64(self_term)
    for r in res.results:
        # [pair, window, subtile, 513] fp16 partials -> per-window minima
        mins = r["outw"].astype(np.float32).min(axis=-1)
        total += mins.astype(np.float64).sum()
    return np.array(np.float32(SCALE * total))
